# revision 19
# baseline (speedup 1.0000x reference)
"""Cosine-sim multi-head attention on 8 trn2 NeuronCores.

Sharding: core c -> (batch b = c//2, head-half hg = c%2). Each core computes
QKV projections for its 6 heads, full attention over S=2048, and a partial
out-projection [S, 768] in fp16. Host sums the two partials per batch + bo.

Per-core pipeline (fp16 operands, fp32 PSUM accum):
  QKV:     hst [768,2048] fp16 x W.T chunks -> q/k/vT [384, 2048]
  norms:   block-ones matmul -> rn; per-head scale folded into rn drain
           (fp32); rsqrt on GPSIMD via pow(rn, -0.5); DRAM-bounce broadcast
  scores:  log2-domain logits s2[j,i] = cos * scale_h * log2(e)
  exp:     2^s2 split ACT (exp(ln2*x), 14/16 chunks) / GPSIMD (pow(2,x) on
           DVE-staged fp16 scores, 2/16 chunks)
  PV-T:    transposed PV, out ctxT[i, dh+1] 65-wide streams; denominator is
           the ones-column; accumulators packed 7+1 across 2 PSUM banks
  norm:    batched reciprocal + per-accum tensor_scalar drain -> ctxTn
  transp:  PE transpose (identity) back to ctx [dh, i]
  outproj: ctx x Wo.T -> o fp16
"""
import numpy as np
import ml_dtypes

import concourse.bass as bass
import concourse.bacc as bacc
import concourse.tile as tile
from concourse import mybir

F16 = mybir.dt.float16
F32 = mybir.dt.float32
EXP = mybir.ActivationFunctionType.Exp
POW = mybir.AluOpType.pow

B, S, D = 4, 2048, 768
H, DH = 12, 64
HPC = 6            # heads per core
NPAIR = 3          # head pairs per core (m-tiles of 128)
NJC = S // 128     # 16 j-chunks
NIC = S // 512     # 4 i-blocks
MAX_LOG_SCALE = float(np.log(1.0 / 0.01))
LN2 = float(np.log(2.0))
LOG2E = float(np.log2(np.e))
POOL_JCS = (5, 10, 15)  # j-chunks whose exp runs on GPSIMD

# ctxT accumulator column offsets: 7 accums at 65-pitch in bank0, 1 in bank1
ACC_OFF = [65 * k for k in range(7)] + [512]

_NC_CACHE = {}


def build_nc():
    nc = bacc.Bacc(None, target_bir_lowering=False, debug=False)

    hst = nc.dram_tensor("hst", [D, S], F16, kind="ExternalInput")
    wqt = nc.dram_tensor("wqt", [D, 384], F16, kind="ExternalInput")
    wkt = nc.dram_tensor("wkt", [D, 384], F16, kind="ExternalInput")
    wvt = nc.dram_tensor("wvt", [D, 384], F16, kind="ExternalInput")
    wot = nc.dram_tensor("wot", [384, D], F16, kind="ExternalInput")
    bq3 = nc.dram_tensor("bq3", [128, 3], F32, kind="ExternalInput")
    bk3 = nc.dram_tensor("bk3", [128, 3], F32, kind="ExternalInput")
    bv3 = nc.dram_tensor("bv3", [128, 3], F32, kind="ExternalInput")
    cq3 = nc.dram_tensor("cq3", [128, 3], F32, kind="ExternalInput")
    i2d = nc.dram_tensor("i2d", [128, 2], F16, kind="ExternalInput")
    idn = nc.dram_tensor("idn", [128, 128], F16, kind="ExternalInput")
    o = nc.dram_tensor("o", [S, D], F16, kind="ExternalOutput")

    with tile.TileContext(nc) as tc:
        import contextlib
        with contextlib.ExitStack() as ctx:
            const = ctx.enter_context(tc.tile_pool(name="const", bufs=1))
            work = ctx.enter_context(tc.tile_pool(name="work", bufs=2, space="PSUM"))
            praw = ctx.enter_context(tc.tile_pool(name="praw", bufs=2))
            kraw_p = ctx.enter_context(tc.tile_pool(name="kraw", bufs=2))
            vtp = ctx.enter_context(tc.tile_pool(name="vtp", bufs=2))
            sqp = ctx.enter_context(tc.tile_pool(name="sqp", bufs=2))
            qsp = ctx.enter_context(tc.tile_pool(name="qsp", bufs=2))
            ksp = ctx.enter_context(tc.tile_pool(name="ksp", bufs=2))
            vap = ctx.enter_context(tc.tile_pool(name="vap", bufs=2))
            rnp = ctx.enter_context(tc.tile_pool(name="rnp", bufs=2))
            rrp = ctx.enter_context(tc.tile_pool(name="rrp", bufs=2))
            bcp = ctx.enter_context(tc.tile_pool(name="bcp", bufs=4))
            cnp = ctx.enter_context(tc.tile_pool(name="cnp", bufs=3))
            ctp = ctx.enter_context(tc.tile_pool(name="ctp", bufs=2))
            rdp = ctx.enter_context(tc.tile_pool(name="rdp", bufs=2))
            dram = ctx.enter_context(tc.tile_pool(name="dram", bufs=4, space="DRAM"))

            # ---- constants ----
            # hst/wq interleaved per-chunk on HWDGE (QKV consumes in kc order);
            # off-critical-path consts go through GPSIMD SWDGE.
            hst_sb = const.tile([128, 6, S], F16)
            w_sbs = [const.tile([128, 6, 384], F16, tag=nm, name=nm)
                     for nm in ("wq", "wk", "wv")]
            for c in range(6):
                nc.sync.dma_start(out=hst_sb[:, c, :],
                                  in_=hst[c * 128:(c + 1) * 128, :])
                nc.scalar.dma_start(out=w_sbs[0][:, c, :],
                                    in_=wqt[c * 128:(c + 1) * 128, :])
            for w_sb, wt in ((w_sbs[1], wkt), (w_sbs[2], wvt)):
                for c in range(6):
                    nc.scalar.dma_start(out=w_sb[:, c, :],
                                        in_=wt[c * 128:(c + 1) * 128, :])
            wot_sb = const.tile([128, 3, D], F16)
            for c in range(3):
                nc.sync.dma_start(out=wot_sb[:, c, :],
                                  in_=wot[c * 128:(c + 1) * 128, :])
            b_sbs = []
            for name, bt in (("bq", bq3), ("bk", bk3), ("bv", bv3)):
                b_sb = const.tile([128, 3], F32, tag=name)
                nc.sync.dma_start(out=b_sb, in_=bt[:, :])
                b_sbs.append(b_sb)
            cq_sb = const.tile([128, 3], F32, tag="cq")
            nc.sync.dma_start(out=cq_sb, in_=cq3[:, :])
            i2_sb = const.tile([128, 2], F16, tag="i2")
            nc.sync.dma_start(out=i2_sb, in_=i2d[:, :])
            id_sb = const.tile([128, 128], F16, tag="idn")
            nc.sync.dma_start(out=id_sb, in_=idn[:, :])
            two16 = const.tile([128, 1024], F16, tag="two16")
            nc.vector.memset(two16, 2.0)
            nh32 = const.tile([128, 1024], F32, tag="nh32")
            nc.vector.memset(nh32, -0.5)

            ctxns = []

            def proj(p, ti):
                w_sb, b_sb = w_sbs[ti], b_sbs[ti]
                pool = (praw, kraw_p, vtp)[ti]
                dest = pool.tile([128, S], F16, tag=f"t{ti}", name=f"t{ti}")
                for ib in range(4):
                    ps = work.tile([128, 512], F32, tag="work", name="qkv_ps")
                    i0 = ib * 512
                    for kc in range(6):
                        nc.tensor.matmul(
                            ps,
                            w_sb[:, kc, p * 128:(p + 1) * 128],
                            hst_sb[:, kc, i0:i0 + 512],
                            start=(kc == 0), stop=(kc == 5))
                    nc.vector.tensor_scalar(
                        out=dest[:, i0:i0 + 512],
                        in0=ps,
                        scalar1=b_sb[:, p:p + 1],
                        scalar2=None,
                        op0=mybir.AluOpType.add)
                return dest

            def qkv_and_norms(p):
                """Project pair p of q/k, norms, then v (v overlaps the
                rsqrt/bounce latency chain on PE)."""
                qraw = proj(p, 0)
                kraw = proj(p, 1)

                # norms^2 via block-ones matmul, M-packed by i-block into 2 banks
                rn_tiles = []
                for bank, src in ((0, qraw), (1, kraw)):
                    rn = work.tile([128, 512], F32, tag="work", name=f"rn{bank}")
                    nc.vector.memset(rn, 1.0)
                    for ib in range(4):
                        sq = sqp.tile([128, 512], F16, tag="sq")
                        nc.vector.tensor_mul(sq, src[:, ib * 512:(ib + 1) * 512],
                                             src[:, ib * 512:(ib + 1) * 512])
                        nc.tensor.matmul(rn[32 * ib:32 * ib + 2, :],
                                         i2_sb, sq,
                                         start=True, stop=True,
                                         tile_position=(0, 32 * ib))
                    rn_tiles.append(rn)
                # drain rn; q rows scaled by c = 1/(scale_h*log2e)^2 in fp32
                rn_sb = rnp.tile([128, 2, 512], F32, tag="rn")
                nc.vector.tensor_scalar(
                    out=rn_sb[:, 0, :], in0=rn_tiles[0],
                    scalar1=cq_sb[:, p:p + 1], scalar2=None,
                    op0=mybir.AluOpType.mult)
                nc.vector.tensor_copy(rn_sb[:, 1, :], rn_tiles[1])
                # rsqrt on GPSIMD: rr = rn ** -0.5   (fp16 out)
                rr = rrp.tile([128, 2, 512], F16, tag="rr")
                nc.gpsimd.tensor_tensor(
                    out=rr.rearrange("p a b -> p (a b)"),
                    in0=rn_sb.rearrange("p a b -> p (a b)"),
                    in1=nh32, op=POW)

                # bounce rows to DRAM, broadcast to rq_bc / rk_bc
                rbs = []
                for bank in range(2):
                    r_dr = dram.tile([4, 2, 512], F16, tag=f"rd{bank}")
                    for ib in range(4):
                        nc.sync.dma_start(out=r_dr[ib, :, :],
                                          in_=rr[32 * ib:32 * ib + 2, bank, :])
                    r_bc = bcp.tile([128, S], F16, tag=f"rb{bank}")
                    for hh in range(2):
                        col = r_dr[:, hh, :]
                        src = bass.AP(tensor=col.tensor, offset=col.offset,
                                      ap=[[0, 64]] + col.ap)
                        nc.sync.dma_start(
                            out=r_bc[hh * 64:(hh + 1) * 64, :].rearrange(
                                "p (a b) -> p a b", a=4),
                            in_=src)
                    rbs.append(r_bc)

                vT = proj(p, 2)

                qs = qsp.tile([128, S], F16, tag="qs")
                nc.vector.tensor_mul(qs, qraw, rbs[0])
                ks = ksp.tile([128, S], F16, tag="ks")
                nc.vector.tensor_mul(ks, kraw, rbs[1])

                return qs, ks, vT

            def build_va(vT):
                # va[p, h, c, 0:64] = v rows for head h, j-chunk c; col 64 = 1
                # One full [128,128] PE transpose per j-chunk gives [j, dh] for
                # both heads at once (head h = cols h*64:(h+1)*64).
                va = vap.tile([128, 2, NJC, 80], F16, tag="va")
                nc.vector.memset(va[:, :, :, 64:65], 1.0)
                for c in range(NJC):
                    tr_ps = work.tile([128, 128], F16, tag="work", name="vtr")
                    nc.tensor.matmul(tr_ps, vT[:, c * 128:(c + 1) * 128],
                                     id_sb, start=True, stop=True,
                                     is_transpose=True)
                    for h in range(2):
                        nc.vector.tensor_copy(va[:, h, c, 0:64],
                                              tr_ps[:, h * 64:(h + 1) * 64])
                return va

            def attention(p, qs, ks, va, scores, epool, stp, cpool, hooks=(),
                          post_t=None):
                ctxTn = ctp.tile([128, 16, 128], F16, tag="ctxTn")
                for ic in range(NIC):
                    for hic, fn in hooks:
                        if ic == hic:
                            fn()
                    i0 = ic * 512
                    # 8 ctxT accums [128, 65] packed 7+1 into 2 banks
                    ctx_ps = cpool.tile([128, 1024], F32, tag="ctx")

                    def pv(jc, e_sb, first, last):
                        for k in range(4):
                            for h in range(2):
                                g = k * 2 + h
                                # start only on the first group per PSUM bank:
                                # start=True zeroing is bank-granular
                                off = ACC_OFF[g]
                                nc.tensor.matmul(
                                    ctx_ps[:, off:off + 65],
                                    e_sb[:, h, k * 128:(k + 1) * 128],
                                    va[:, h, jc, 0:65],
                                    start=(first and g in (0, 7)),
                                    stop=last,
                                    skip_group_check=True)

                    def scores_mm(jc):
                        s_ps = scores.tile([128, 2, 512], F32, tag="s")
                        nc.tensor.matmul(s_ps[:, 0, :],
                                         ks[0:64, jc * 128:(jc + 1) * 128],
                                         qs[0:64, i0:i0 + 512],
                                         start=True, stop=True, tile_position=(0, 0))
                        nc.tensor.matmul(s_ps[:, 1, :],
                                         ks[64:128, jc * 128:(jc + 1) * 128],
                                         qs[64:128, i0:i0 + 512],
                                         start=True, stop=True, tile_position=(64, 0))
                        return s_ps

                    # Pool-share chunks: scores emitted first (their exp runs
                    # on GPSIMD off DVE-staged copies), PV emitted last so the
                    # in-order PE queue never head-blocks on the slow path.
                    pool_es = []
                    for jc in POOL_JCS:
                        s_ps = scores_mm(jc)
                        st = stp.tile([128, 2, 512], F16, tag="st")
                        nc.vector.tensor_copy(
                            st.rearrange("p a b -> p (a b)"),
                            s_ps.rearrange("p a b -> p (a b)"))
                        e_sb = epool.tile([128, 2, 512], F16, tag="e")
                        nc.gpsimd.tensor_tensor(
                            out=e_sb.rearrange("p a b -> p (a b)"),
                            in0=two16,
                            in1=st.rearrange("p a b -> p (a b)"),
                            op=POW)
                        pool_es.append((jc, e_sb))
                    act_jcs = [jc for jc in range(NJC) if jc not in POOL_JCS]
                    for n, jc in enumerate(act_jcs):
                        s_ps = scores_mm(jc)
                        e_sb = epool.tile([128, 2, 512], F16, tag="e")
                        nc.scalar.activation(
                            e_sb.rearrange("p a b -> p (a b)"),
                            s_ps.rearrange("p a b -> p (a b)"),
                            EXP, scale=LN2)
                        pv(jc, e_sb, first=(n == 0), last=False)
                    for n, (jc, e_sb) in enumerate(pool_es):
                        pv(jc, e_sb, first=False, last=(n == len(pool_es) - 1))
                    # normalize: rden = 1/denom-col, drain accums to ctxTn
                    rden = rdp.tile([128, 8], F32, tag="rden")
                    dsrc = bass.AP(tensor=ctx_ps.tensor, offset=ctx_ps.offset + 64,
                                   ap=[ctx_ps.ap[0], [65, 7]])
                    nc.vector.reciprocal(rden[:, 0:7], dsrc)
                    nc.vector.reciprocal(rden[:, 7:8], ctx_ps[:, 576:577])
                    for k in range(4):
                        for h in range(2):
                            off = ACC_OFF[k * 2 + h]
                            nc.vector.tensor_scalar(
                                out=ctxTn[:, ic * 4 + k, h * 64:(h + 1) * 64],
                                in0=ctx_ps[:, off:off + 64],
                                scalar1=rden[:, k * 2 + h:k * 2 + h + 1],
                                scalar2=None,
                                op0=mybir.AluOpType.mult)
                # PE transpose back to [dh, i]
                ctxn = cnp.tile([128, S], F16, tag="ctxn")
                for t in range(16):
                    tr_ps = work.tile([128, 128], F16, tag="work", name="tr")
                    nc.tensor.matmul(tr_ps, ctxTn[:, t, :], id_sb,
                                     start=True, stop=True, is_transpose=True)
                    nc.vector.tensor_copy(ctxn[:, t * 128:(t + 1) * 128], tr_ps)
                    if post_t is not None:
                        post_t(t, ctxn)
                return ctxn

            with tc.tile_pool(name="scores", bufs=2, space="PSUM") as scores, \
                 tc.tile_pool(name="epool", bufs=8) as epool, \
                 tc.tile_pool(name="stp", bufs=2) as stp, \
                 tc.tile_pool(name="cpool", bufs=1, space="PSUM") as cpool:
                pending = {}
                qs0, ks0, vT0 = qkv_and_norms(0)
                pending[0] = (qs0, ks0, build_va(vT0))

                def make_hooks(pn):
                    part = {}

                    def h1():
                        part["qkv"] = qkv_and_norms(pn)

                    def h2():
                        qs_, ks_, vT_ = part["qkv"]
                        pending[pn] = (qs_, ks_, build_va(vT_))
                    return ((1, h1), (3, h2))

                with tc.tile_pool(name="osb", bufs=3) as osb:
                    def outproj_st(st, ctxn2):
                        o_sb = osb.tile([128, D], F16, tag="osb")
                        pairs = [ctxns[0], ctxns[1], ctxn2]
                        for nn in range(2):
                            o_ps = work.tile([128, 512], F32, tag="work",
                                             name="o_ps")
                            for p in range(NPAIR):
                                nc.tensor.matmul(
                                    o_ps[:, 0:384],
                                    pairs[p][:, st * 128:(st + 1) * 128],
                                    wot_sb[:, p, nn * 384:(nn + 1) * 384],
                                    start=(p == 0), stop=(p == NPAIR - 1))
                            nc.vector.tensor_copy(
                                o_sb[:, nn * 384:(nn + 1) * 384], o_ps[:, 0:384])
                        nc.sync.dma_start(out=o[st * 128:(st + 1) * 128, :],
                                          in_=o_sb)

                    for p in range(NPAIR):
                        qs, ks, va = pending.pop(p)
                        hooks = make_hooks(p + 1) if p + 1 < NPAIR else ()
                        post = outproj_st if p == NPAIR - 1 else None
                        ctxns.append(attention(p, qs, ks, va, scores, epool,
                                               stp, cpool, hooks=hooks,
                                               post_t=post))

    nc.compile()
    return nc


def _prep_core_inputs(inputs, b, hg):
    f16 = np.float16
    hs = inputs["hidden_states"]
    rows = slice(hg * 384, (hg + 1) * 384)
    scale6 = np.exp(np.minimum(
        inputs["logit_scale"].reshape(H)[hg * HPC:(hg + 1) * HPC],
        MAX_LOG_SCALE)).astype(np.float64)

    def b3(bias):
        return np.ascontiguousarray(bias[rows].reshape(3, 128).T).astype(np.float32)

    # cq: per-partition scale c = 1/(scale_h*log2e)^2 applied to rn_q
    cq = np.ones((128, 3), np.float32)
    for p in range(3):
        for ib in range(4):
            for hh in range(2):
                cq[32 * ib + hh, p] = 1.0 / (scale6[p * 2 + hh] * LOG2E) ** 2
    i2 = np.zeros((128, 2), f16)
    i2[0:64, 0] = 1.0
    i2[64:128, 1] = 1.0
    return {
        "hst": np.ascontiguousarray(hs[b].T).astype(f16),
        "wqt": np.ascontiguousarray(inputs["Wq"][rows].T).astype(f16),
        "wkt": np.ascontiguousarray(inputs["Wk"][rows].T).astype(f16),
        "wvt": np.ascontiguousarray(inputs["Wv"][rows].T).astype(f16),
        "wot": np.ascontiguousarray(inputs["Wo"][:, rows].T).astype(f16),
        "bq3": b3(inputs["bq"]),
        "bk3": b3(inputs["bk"]),
        "bv3": b3(inputs["bv"]),
        "cq3": cq,
        "i2d": i2,
        "idn": np.eye(128, dtype=f16),
    }


def kernel(**inputs):
    from concourse.bass_utils import run_bass_kernel_spmd
    inputs = {k: np.asarray(v) for k, v in inputs.items()}
    if "nc" not in _NC_CACHE:
        _NC_CACHE["nc"] = build_nc()
    nc = _NC_CACHE["nc"]
    in_maps = [_prep_core_inputs(inputs, c // 2, c % 2) for c in range(8)]
    res = run_bass_kernel_spmd(nc, in_maps, core_ids=list(range(8)))
    out = np.empty((B, S, D), np.float32)
    bo = inputs["bo"].astype(np.float32)
    for b in range(B):
        out[b] = (res.results[2 * b]["o"].astype(np.float32)
                  + res.results[2 * b + 1]["o"].astype(np.float32) + bo)
    return out


# revision 20
# speedup vs baseline: 1.0197x; 1.0197x over previous
"""Cosine-sim multi-head attention on 8 trn2 NeuronCores.

Sharding: core c -> (batch b = c//2, head-half hg = c%2). Each core computes
QKV projections for its 6 heads, full attention over S=2048, and a partial
out-projection [S, 768] in fp16. Host sums the two partials per batch + bo.

Per-core pipeline (fp16 operands, fp32 PSUM accum):
  QKV:     hst [768,2048] fp16 x W.T chunks -> q/k/vT [384, 2048]
  norms:   block-ones matmul -> rn; per-head scale folded into rn drain
           (fp32); rsqrt on GPSIMD via pow(rn, -0.5); DRAM-bounce broadcast
  scores:  log2-domain logits s2[j,i] = cos * scale_h * log2(e)
  exp:     2^s2 split ACT (exp(ln2*x), 14/16 chunks) / GPSIMD (pow(2,x) on
           DVE-staged fp16 scores, 2/16 chunks)
  PV-T:    transposed PV, out ctxT[i, dh+1] 65-wide streams; denominator is
           the ones-column; accumulators packed 7+1 across 2 PSUM banks
  norm:    batched reciprocal + per-accum tensor_scalar drain -> ctxTn
  transp:  PE transpose (identity) back to ctx [dh, i]
  outproj: ctx x Wo.T -> o fp16
"""
import numpy as np
import ml_dtypes

import concourse.bass as bass
import concourse.bacc as bacc
import concourse.tile as tile
from concourse import mybir

F16 = mybir.dt.float16
F32 = mybir.dt.float32
EXP = mybir.ActivationFunctionType.Exp
POW = mybir.AluOpType.pow

B, S, D = 4, 2048, 768
H, DH = 12, 64
HPC = 6            # heads per core
NPAIR = 3          # head pairs per core (m-tiles of 128)
NJC = S // 128     # 16 j-chunks
NIC = S // 512     # 4 i-blocks
MAX_LOG_SCALE = float(np.log(1.0 / 0.01))
LN2 = float(np.log(2.0))
LOG2E = float(np.log2(np.e))
POOL_JCS = (6, 13)  # j-chunks whose exp runs on GPSIMD

# ctxT accumulator column offsets: 7 accums at 65-pitch in bank0, 1 in bank1
ACC_OFF = [65 * k for k in range(7)] + [512]

_NC_CACHE = {}


def build_nc():
    nc = bacc.Bacc(None, target_bir_lowering=False, debug=False)

    hst = nc.dram_tensor("hst", [D, S], F16, kind="ExternalInput")
    wqt = nc.dram_tensor("wqt", [D, 384], F16, kind="ExternalInput")
    wkt = nc.dram_tensor("wkt", [D, 384], F16, kind="ExternalInput")
    wvt = nc.dram_tensor("wvt", [D, 384], F16, kind="ExternalInput")
    wot = nc.dram_tensor("wot", [384, D], F16, kind="ExternalInput")
    bq3 = nc.dram_tensor("bq3", [128, 3], F32, kind="ExternalInput")
    bk3 = nc.dram_tensor("bk3", [128, 3], F32, kind="ExternalInput")
    bv3 = nc.dram_tensor("bv3", [128, 3], F32, kind="ExternalInput")
    cq3 = nc.dram_tensor("cq3", [128, 3], F32, kind="ExternalInput")
    i2d = nc.dram_tensor("i2d", [128, 2], F16, kind="ExternalInput")
    idn = nc.dram_tensor("idn", [128, 128], F16, kind="ExternalInput")
    o = nc.dram_tensor("o", [S, D], F16, kind="ExternalOutput")

    with tile.TileContext(nc) as tc:
        import contextlib
        with contextlib.ExitStack() as ctx:
            const = ctx.enter_context(tc.tile_pool(name="const", bufs=1))
            work = ctx.enter_context(tc.tile_pool(name="work", bufs=2, space="PSUM"))
            praw = ctx.enter_context(tc.tile_pool(name="praw", bufs=2))
            kraw_p = ctx.enter_context(tc.tile_pool(name="kraw", bufs=2))
            vtp = ctx.enter_context(tc.tile_pool(name="vtp", bufs=2))
            sqp = ctx.enter_context(tc.tile_pool(name="sqp", bufs=2))
            qsp = ctx.enter_context(tc.tile_pool(name="qsp", bufs=2))
            ksp = ctx.enter_context(tc.tile_pool(name="ksp", bufs=2))
            vap = ctx.enter_context(tc.tile_pool(name="vap", bufs=2))
            rnp = ctx.enter_context(tc.tile_pool(name="rnp", bufs=2))
            rrp = ctx.enter_context(tc.tile_pool(name="rrp", bufs=2))
            bcp = ctx.enter_context(tc.tile_pool(name="bcp", bufs=4))
            cnp = ctx.enter_context(tc.tile_pool(name="cnp", bufs=3))
            ctp = ctx.enter_context(tc.tile_pool(name="ctp", bufs=2))
            rdp = ctx.enter_context(tc.tile_pool(name="rdp", bufs=2))
            dram = ctx.enter_context(tc.tile_pool(name="dram", bufs=4, space="DRAM"))

            # ---- constants ----
            # hst/wq interleaved per-chunk on HWDGE (QKV consumes in kc order);
            # off-critical-path consts go through GPSIMD SWDGE.
            hst_sb = const.tile([128, 6, S], F16)
            w_sbs = [const.tile([128, 6, 384], F16, tag=nm, name=nm)
                     for nm in ("wq", "wk", "wv")]
            for c in range(6):
                nc.sync.dma_start(out=hst_sb[:, c, :],
                                  in_=hst[c * 128:(c + 1) * 128, :])
                nc.scalar.dma_start(out=w_sbs[0][:, c, :],
                                    in_=wqt[c * 128:(c + 1) * 128, :])
            for w_sb, wt in ((w_sbs[1], wkt), (w_sbs[2], wvt)):
                for c in range(6):
                    nc.scalar.dma_start(out=w_sb[:, c, :],
                                        in_=wt[c * 128:(c + 1) * 128, :])
            wot_sb = const.tile([128, 3, D], F16)
            for c in range(3):
                nc.sync.dma_start(out=wot_sb[:, c, :],
                                  in_=wot[c * 128:(c + 1) * 128, :])
            b_sbs = []
            for name, bt in (("bq", bq3), ("bk", bk3), ("bv", bv3)):
                b_sb = const.tile([128, 3], F32, tag=name)
                nc.sync.dma_start(out=b_sb, in_=bt[:, :])
                b_sbs.append(b_sb)
            cq_sb = const.tile([128, 3], F32, tag="cq")
            nc.sync.dma_start(out=cq_sb, in_=cq3[:, :])
            i2_sb = const.tile([128, 2], F16, tag="i2")
            nc.sync.dma_start(out=i2_sb, in_=i2d[:, :])
            id_sb = const.tile([128, 128], F16, tag="idn")
            nc.sync.dma_start(out=id_sb, in_=idn[:, :])
            two16 = const.tile([128, 1024], F16, tag="two16")
            nc.vector.memset(two16, 2.0)
            nh32 = const.tile([128, 1024], F32, tag="nh32")
            nc.vector.memset(nh32, -0.5)

            ctxns = []

            def proj(p, ti):
                w_sb, b_sb = w_sbs[ti], b_sbs[ti]
                pool = (praw, kraw_p, vtp)[ti]
                dest = pool.tile([128, S], F16, tag=f"t{ti}", name=f"t{ti}")
                for ib in range(4):
                    ps = work.tile([128, 512], F32, tag="work", name="qkv_ps")
                    i0 = ib * 512
                    for kc in range(6):
                        nc.tensor.matmul(
                            ps,
                            w_sb[:, kc, p * 128:(p + 1) * 128],
                            hst_sb[:, kc, i0:i0 + 512],
                            start=(kc == 0), stop=(kc == 5))
                    nc.vector.tensor_scalar(
                        out=dest[:, i0:i0 + 512],
                        in0=ps,
                        scalar1=b_sb[:, p:p + 1],
                        scalar2=None,
                        op0=mybir.AluOpType.add)
                return dest

            def qkv_and_norms(p):
                """Project pair p of q/k, norms, then v (v overlaps the
                rsqrt/bounce latency chain on PE)."""
                qraw = proj(p, 0)
                kraw = proj(p, 1)

                # norms^2 via block-ones matmul, M-packed by i-block into 2 banks
                rn_tiles = []
                for bank, src in ((0, qraw), (1, kraw)):
                    rn = work.tile([128, 512], F32, tag="work", name=f"rn{bank}")
                    nc.vector.memset(rn, 1.0)
                    for ib in range(4):
                        sq = sqp.tile([128, 512], F16, tag="sq")
                        nc.vector.tensor_mul(sq, src[:, ib * 512:(ib + 1) * 512],
                                             src[:, ib * 512:(ib + 1) * 512])
                        nc.tensor.matmul(rn[32 * ib:32 * ib + 2, :],
                                         i2_sb, sq,
                                         start=True, stop=True,
                                         tile_position=(0, 32 * ib))
                    rn_tiles.append(rn)
                # drain rn; q rows scaled by c = 1/(scale_h*log2e)^2 in fp32
                rn_sb = rnp.tile([128, 2, 512], F32, tag="rn")
                nc.vector.tensor_scalar(
                    out=rn_sb[:, 0, :], in0=rn_tiles[0],
                    scalar1=cq_sb[:, p:p + 1], scalar2=None,
                    op0=mybir.AluOpType.mult)
                nc.vector.tensor_copy(rn_sb[:, 1, :], rn_tiles[1])
                # rsqrt on GPSIMD: rr = rn ** -0.5   (fp16 out)
                rr = rrp.tile([128, 2, 512], F16, tag="rr")
                nc.gpsimd.tensor_tensor(
                    out=rr.rearrange("p a b -> p (a b)"),
                    in0=rn_sb.rearrange("p a b -> p (a b)"),
                    in1=nh32, op=POW)

                # bounce rows to DRAM, broadcast to rq_bc / rk_bc
                rbs = []
                for bank in range(2):
                    r_dr = dram.tile([4, 2, 512], F16, tag=f"rd{bank}")
                    for ib in range(4):
                        nc.sync.dma_start(out=r_dr[ib, :, :],
                                          in_=rr[32 * ib:32 * ib + 2, bank, :])
                    r_bc = bcp.tile([128, S], F16, tag=f"rb{bank}")
                    for hh in range(2):
                        col = r_dr[:, hh, :]
                        src = bass.AP(tensor=col.tensor, offset=col.offset,
                                      ap=[[0, 64]] + col.ap)
                        nc.sync.dma_start(
                            out=r_bc[hh * 64:(hh + 1) * 64, :].rearrange(
                                "p (a b) -> p a b", a=4),
                            in_=src)
                    rbs.append(r_bc)

                vT = proj(p, 2)

                qs = qsp.tile([128, S], F16, tag="qs")
                nc.vector.tensor_mul(qs, qraw, rbs[0])
                ks = ksp.tile([128, S], F16, tag="ks")
                nc.vector.tensor_mul(ks, kraw, rbs[1])

                return qs, ks, vT

            def build_va(vT):
                # va[p, h, c, 0:64] = v rows for head h, j-chunk c; col 64 = 1
                # One full [128,128] PE transpose per j-chunk gives [j, dh] for
                # both heads at once (head h = cols h*64:(h+1)*64).
                va = vap.tile([128, 2, NJC, 80], F16, tag="va")
                nc.vector.memset(va[:, :, :, 64:65], 1.0)
                for c in range(NJC):
                    tr_ps = work.tile([128, 128], F16, tag="work", name="vtr")
                    nc.tensor.matmul(tr_ps, vT[:, c * 128:(c + 1) * 128],
                                     id_sb, start=True, stop=True,
                                     is_transpose=True)
                    for h in range(2):
                        nc.vector.tensor_copy(va[:, h, c, 0:64],
                                              tr_ps[:, h * 64:(h + 1) * 64])
                return va

            def attention(p, qs, ks, va, scores, epool, stp, cpool, hooks=(),
                          post_t=None):
                ctxTn = ctp.tile([128, 16, 128], F16, tag="ctxTn")
                for ic in range(NIC):
                    for hic, fn in hooks:
                        if ic == hic:
                            fn()
                    i0 = ic * 512
                    # 8 ctxT accums [128, 65] packed 7+1 into 2 banks
                    ctx_ps = cpool.tile([128, 1024], F32, tag="ctx")

                    def pv(jc, e_sb, first, last):
                        for k in range(4):
                            for h in range(2):
                                g = k * 2 + h
                                # start only on the first group per PSUM bank:
                                # start=True zeroing is bank-granular
                                off = ACC_OFF[g]
                                nc.tensor.matmul(
                                    ctx_ps[:, off:off + 65],
                                    e_sb[:, h, k * 128:(k + 1) * 128],
                                    va[:, h, jc, 0:65],
                                    start=(first and g in (0, 7)),
                                    stop=last,
                                    skip_group_check=True)

                    def scores_mm(jc):
                        s_ps = scores.tile([128, 2, 512], F32, tag="s")
                        nc.tensor.matmul(s_ps[:, 0, :],
                                         ks[0:64, jc * 128:(jc + 1) * 128],
                                         qs[0:64, i0:i0 + 512],
                                         start=True, stop=True, tile_position=(0, 0))
                        nc.tensor.matmul(s_ps[:, 1, :],
                                         ks[64:128, jc * 128:(jc + 1) * 128],
                                         qs[64:128, i0:i0 + 512],
                                         start=True, stop=True, tile_position=(64, 0))
                        return s_ps

                    # Pool-share chunks: scores emitted first (their exp runs
                    # on GPSIMD off DVE-staged copies), PV emitted last so the
                    # in-order PE queue never head-blocks on the slow path.
                    pool_es = []
                    for jc in POOL_JCS:
                        s_ps = scores_mm(jc)
                        st = stp.tile([128, 2, 512], F16, tag="st")
                        nc.vector.tensor_copy(
                            st.rearrange("p a b -> p (a b)"),
                            s_ps.rearrange("p a b -> p (a b)"))
                        e_sb = epool.tile([128, 2, 512], F16, tag="e")
                        nc.gpsimd.tensor_tensor(
                            out=e_sb.rearrange("p a b -> p (a b)"),
                            in0=two16,
                            in1=st.rearrange("p a b -> p (a b)"),
                            op=POW)
                        pool_es.append((jc, e_sb))
                    act_jcs = [jc for jc in range(NJC) if jc not in POOL_JCS]
                    for n, jc in enumerate(act_jcs):
                        s_ps = scores_mm(jc)
                        e_sb = epool.tile([128, 2, 512], F16, tag="e")
                        nc.scalar.activation(
                            e_sb.rearrange("p a b -> p (a b)"),
                            s_ps.rearrange("p a b -> p (a b)"),
                            EXP, scale=LN2)
                        pv(jc, e_sb, first=(n == 0), last=False)
                    for n, (jc, e_sb) in enumerate(pool_es):
                        pv(jc, e_sb, first=False, last=(n == len(pool_es) - 1))
                    # normalize: rden = 1/denom-col, drain accums to ctxTn
                    rden = rdp.tile([128, 8], F32, tag="rden")
                    dsrc = bass.AP(tensor=ctx_ps.tensor, offset=ctx_ps.offset + 64,
                                   ap=[ctx_ps.ap[0], [65, 7]])
                    nc.vector.reciprocal(rden[:, 0:7], dsrc)
                    nc.vector.reciprocal(rden[:, 7:8], ctx_ps[:, 576:577])
                    for k in range(4):
                        for h in range(2):
                            off = ACC_OFF[k * 2 + h]
                            nc.vector.tensor_scalar(
                                out=ctxTn[:, ic * 4 + k, h * 64:(h + 1) * 64],
                                in0=ctx_ps[:, off:off + 64],
                                scalar1=rden[:, k * 2 + h:k * 2 + h + 1],
                                scalar2=None,
                                op0=mybir.AluOpType.mult)
                # PE transpose back to [dh, i]
                ctxn = cnp.tile([128, S], F16, tag="ctxn")
                for t in range(16):
                    tr_ps = work.tile([128, 128], F16, tag="work", name="tr")
                    nc.tensor.matmul(tr_ps, ctxTn[:, t, :], id_sb,
                                     start=True, stop=True, is_transpose=True)
                    nc.vector.tensor_copy(ctxn[:, t * 128:(t + 1) * 128], tr_ps)
                    if post_t is not None:
                        post_t(t, ctxn)
                return ctxn

            with tc.tile_pool(name="scores", bufs=2, space="PSUM") as scores, \
                 tc.tile_pool(name="epool", bufs=8) as epool, \
                 tc.tile_pool(name="stp", bufs=2) as stp, \
                 tc.tile_pool(name="cpool", bufs=1, space="PSUM") as cpool:
                pending = {}
                qs0, ks0, vT0 = qkv_and_norms(0)
                pending[0] = (qs0, ks0, build_va(vT0))

                def make_hooks(pn):
                    part = {}

                    def h1():
                        part["qkv"] = qkv_and_norms(pn)

                    def h2():
                        qs_, ks_, vT_ = part["qkv"]
                        pending[pn] = (qs_, ks_, build_va(vT_))
                    return ((1, h1), (3, h2))

                with tc.tile_pool(name="osb", bufs=3) as osb:
                    def outproj_st(st, ctxn2):
                        o_sb = osb.tile([128, D], F16, tag="osb")
                        pairs = [ctxns[0], ctxns[1], ctxn2]
                        for nn in range(2):
                            o_ps = work.tile([128, 512], F32, tag="work",
                                             name="o_ps")
                            for p in range(NPAIR):
                                nc.tensor.matmul(
                                    o_ps[:, 0:384],
                                    pairs[p][:, st * 128:(st + 1) * 128],
                                    wot_sb[:, p, nn * 384:(nn + 1) * 384],
                                    start=(p == 0), stop=(p == NPAIR - 1))
                            nc.vector.tensor_copy(
                                o_sb[:, nn * 384:(nn + 1) * 384], o_ps[:, 0:384])
                        nc.sync.dma_start(out=o[st * 128:(st + 1) * 128, :],
                                          in_=o_sb)

                    for p in range(NPAIR):
                        qs, ks, va = pending.pop(p)
                        hooks = make_hooks(p + 1) if p + 1 < NPAIR else ()
                        post = outproj_st if p == NPAIR - 1 else None
                        ctxns.append(attention(p, qs, ks, va, scores, epool,
                                               stp, cpool, hooks=hooks,
                                               post_t=post))

    nc.compile()
    return nc


def _prep_core_inputs(inputs, b, hg):
    f16 = np.float16
    hs = inputs["hidden_states"]
    rows = slice(hg * 384, (hg + 1) * 384)
    scale6 = np.exp(np.minimum(
        inputs["logit_scale"].reshape(H)[hg * HPC:(hg + 1) * HPC],
        MAX_LOG_SCALE)).astype(np.float64)

    def b3(bias):
        return np.ascontiguousarray(bias[rows].reshape(3, 128).T).astype(np.float32)

    # cq: per-partition scale c = 1/(scale_h*log2e)^2 applied to rn_q
    cq = np.ones((128, 3), np.float32)
    for p in range(3):
        for ib in range(4):
            for hh in range(2):
                cq[32 * ib + hh, p] = 1.0 / (scale6[p * 2 + hh] * LOG2E) ** 2
    i2 = np.zeros((128, 2), f16)
    i2[0:64, 0] = 1.0
    i2[64:128, 1] = 1.0
    return {
        "hst": np.ascontiguousarray(hs[b].T).astype(f16),
        "wqt": np.ascontiguousarray(inputs["Wq"][rows].T).astype(f16),
        "wkt": np.ascontiguousarray(inputs["Wk"][rows].T).astype(f16),
        "wvt": np.ascontiguousarray(inputs["Wv"][rows].T).astype(f16),
        "wot": np.ascontiguousarray(inputs["Wo"][:, rows].T).astype(f16),
        "bq3": b3(inputs["bq"]),
        "bk3": b3(inputs["bk"]),
        "bv3": b3(inputs["bv"]),
        "cq3": cq,
        "i2d": i2,
        "idn": np.eye(128, dtype=f16),
    }


def kernel(**inputs):
    from concourse.bass_utils import run_bass_kernel_spmd
    inputs = {k: np.asarray(v) for k, v in inputs.items()}
    if "nc" not in _NC_CACHE:
        _NC_CACHE["nc"] = build_nc()
    nc = _NC_CACHE["nc"]
    in_maps = [_prep_core_inputs(inputs, c // 2, c % 2) for c in range(8)]
    res = run_bass_kernel_spmd(nc, in_maps, core_ids=list(range(8)))
    out = np.empty((B, S, D), np.float32)
    bo = inputs["bo"].astype(np.float32)
    for b in range(B):
        out[b] = (res.results[2 * b]["o"].astype(np.float32)
                  + res.results[2 * b + 1]["o"].astype(np.float32) + bo)
    return out


# revision 21
# speedup vs baseline: 1.0236x; 1.0038x over previous
"""Cosine-sim multi-head attention on 8 trn2 NeuronCores.

Sharding: core c -> (batch b = c//2, head-half hg = c%2). Each core computes
QKV projections for its 6 heads, full attention over S=2048, and a partial
out-projection [S, 768] in fp16. Host sums the two partials per batch + bo.

Per-core pipeline (fp16 operands, fp32 PSUM accum):
  QKV:     hst [768,2048] fp16 x W.T chunks -> q/k/vT [384, 2048]
  norms:   block-ones matmul -> rn; per-head scale folded into rn drain
           (fp32); rsqrt on GPSIMD via pow(rn, -0.5); DRAM-bounce broadcast
  scores:  log2-domain logits s2[j,i] = cos * scale_h * log2(e)
  exp:     2^s2 split ACT (exp(ln2*x), 14/16 chunks) / GPSIMD (pow(2,x) on
           DVE-staged fp16 scores, 2/16 chunks)
  PV-T:    transposed PV, out ctxT[i, dh+1] 65-wide streams; denominator is
           the ones-column; accumulators packed 7+1 across 2 PSUM banks
  norm:    batched reciprocal + per-accum tensor_scalar drain -> ctxTn
  transp:  PE transpose (identity) back to ctx [dh, i]
  outproj: ctx x Wo.T -> o fp16
"""
import numpy as np
import ml_dtypes

import concourse.bass as bass
import concourse.bacc as bacc
import concourse.tile as tile
from concourse import mybir

F16 = mybir.dt.float16
F32 = mybir.dt.float32
EXP = mybir.ActivationFunctionType.Exp
POW = mybir.AluOpType.pow

B, S, D = 4, 2048, 768
H, DH = 12, 64
HPC = 6            # heads per core
NPAIR = 3          # head pairs per core (m-tiles of 128)
NJC = S // 128     # 16 j-chunks
NIC = S // 512     # 4 i-blocks
MAX_LOG_SCALE = float(np.log(1.0 / 0.01))
LN2 = float(np.log(2.0))
LOG2E = float(np.log2(np.e))
POOL_JCS = (6, 13)  # j-chunks whose exp runs on GPSIMD

# ctxT accumulator column offsets: 7 accums at 65-pitch in bank0, 1 in bank1
ACC_OFF = [65 * k for k in range(7)] + [512]

_NC_CACHE = {}


def build_nc():
    nc = bacc.Bacc(None, target_bir_lowering=False, debug=False)

    hst = nc.dram_tensor("hst", [D, S], F16, kind="ExternalInput")
    wqt = nc.dram_tensor("wqt", [D, 384], F16, kind="ExternalInput")
    wkt = nc.dram_tensor("wkt", [D, 384], F16, kind="ExternalInput")
    wvt = nc.dram_tensor("wvt", [D, 384], F16, kind="ExternalInput")
    wot = nc.dram_tensor("wot", [384, D], F16, kind="ExternalInput")
    bq3 = nc.dram_tensor("bq3", [128, 3], F32, kind="ExternalInput")
    bk3 = nc.dram_tensor("bk3", [128, 3], F32, kind="ExternalInput")
    bv3 = nc.dram_tensor("bv3", [128, 3], F32, kind="ExternalInput")
    cq3 = nc.dram_tensor("cq3", [128, 3], F32, kind="ExternalInput")
    i2d = nc.dram_tensor("i2d", [128, 2], F16, kind="ExternalInput")
    idn = nc.dram_tensor("idn", [128, 128], F16, kind="ExternalInput")
    o = nc.dram_tensor("o", [S, D], F16, kind="ExternalOutput")

    with tile.TileContext(nc) as tc:
        import contextlib
        with contextlib.ExitStack() as ctx:
            const = ctx.enter_context(tc.tile_pool(name="const", bufs=1))
            work = ctx.enter_context(tc.tile_pool(name="work", bufs=2, space="PSUM"))
            praw = ctx.enter_context(tc.tile_pool(name="praw", bufs=2))
            kraw_p = ctx.enter_context(tc.tile_pool(name="kraw", bufs=2))
            vtp = ctx.enter_context(tc.tile_pool(name="vtp", bufs=2))
            sqp = ctx.enter_context(tc.tile_pool(name="sqp", bufs=2))
            qsp = ctx.enter_context(tc.tile_pool(name="qsp", bufs=2))
            ksp = ctx.enter_context(tc.tile_pool(name="ksp", bufs=2))
            vap = ctx.enter_context(tc.tile_pool(name="vap", bufs=2))
            rnp = ctx.enter_context(tc.tile_pool(name="rnp", bufs=2))
            rrp = ctx.enter_context(tc.tile_pool(name="rrp", bufs=2))
            bcp = ctx.enter_context(tc.tile_pool(name="bcp", bufs=4))
            cnp = ctx.enter_context(tc.tile_pool(name="cnp", bufs=3))
            ctp = ctx.enter_context(tc.tile_pool(name="ctp", bufs=2))
            rdp = ctx.enter_context(tc.tile_pool(name="rdp", bufs=2))
            dram = ctx.enter_context(tc.tile_pool(name="dram", bufs=4, space="DRAM"))

            # ---- constants ----
            # hst/wq interleaved per-chunk on HWDGE (QKV consumes in kc order);
            # off-critical-path consts go through GPSIMD SWDGE.
            hst_sb = const.tile([128, 6, S], F16)
            w_sbs = [const.tile([128, 6, 384], F16, tag=nm, name=nm)
                     for nm in ("wq", "wk", "wv")]
            for c in range(6):
                nc.sync.dma_start(out=hst_sb[:, c, :],
                                  in_=hst[c * 128:(c + 1) * 128, :])
                nc.scalar.dma_start(out=w_sbs[0][:, c, :],
                                    in_=wqt[c * 128:(c + 1) * 128, :])
            for w_sb, wt in ((w_sbs[1], wkt), (w_sbs[2], wvt)):
                for c in range(6):
                    nc.scalar.dma_start(out=w_sb[:, c, :],
                                        in_=wt[c * 128:(c + 1) * 128, :])
            wot_sb = const.tile([128, 3, D], F16)
            for c in range(3):
                nc.sync.dma_start(out=wot_sb[:, c, :],
                                  in_=wot[c * 128:(c + 1) * 128, :])
            b_sbs = []
            for name, bt in (("bq", bq3), ("bk", bk3), ("bv", bv3)):
                b_sb = const.tile([128, 3], F32, tag=name)
                nc.sync.dma_start(out=b_sb, in_=bt[:, :])
                b_sbs.append(b_sb)
            cq_sb = const.tile([128, 3], F32, tag="cq")
            nc.sync.dma_start(out=cq_sb, in_=cq3[:, :])
            i2_sb = const.tile([128, 2], F16, tag="i2")
            nc.sync.dma_start(out=i2_sb, in_=i2d[:, :])
            id_sb = const.tile([128, 128], F16, tag="idn")
            nc.sync.dma_start(out=id_sb, in_=idn[:, :])
            two16 = const.tile([128, 1024], F16, tag="two16")
            nc.vector.memset(two16, 2.0)
            nh32 = const.tile([128, 1024], F32, tag="nh32")
            nc.vector.memset(nh32, -0.5)

            ctxns = []

            def proj(p, ti):
                w_sb, b_sb = w_sbs[ti], b_sbs[ti]
                pool = (praw, kraw_p, vtp)[ti]
                dest = pool.tile([128, S], F16, tag=f"t{ti}", name=f"t{ti}")
                for ib in range(4):
                    ps = work.tile([128, 512], F32, tag="work", name="qkv_ps")
                    i0 = ib * 512
                    for kc in range(6):
                        nc.tensor.matmul(
                            ps,
                            w_sb[:, kc, p * 128:(p + 1) * 128],
                            hst_sb[:, kc, i0:i0 + 512],
                            start=(kc == 0), stop=(kc == 5))
                    nc.vector.tensor_scalar(
                        out=dest[:, i0:i0 + 512],
                        in0=ps,
                        scalar1=b_sb[:, p:p + 1],
                        scalar2=None,
                        op0=mybir.AluOpType.add)
                return dest

            def qkv_and_norms(p):
                """Project pair p of q/k/v, then norms."""
                qraw = proj(p, 0)
                kraw = proj(p, 1)
                vT = proj(p, 2)

                # norms^2 via block-ones matmul, M-packed by i-block into 2 banks
                rn_tiles = []
                for bank, src in ((0, qraw), (1, kraw)):
                    rn = work.tile([128, 512], F32, tag="work", name=f"rn{bank}")
                    nc.vector.memset(rn, 1.0)
                    for ib in range(4):
                        sq = sqp.tile([128, 512], F16, tag="sq")
                        nc.vector.tensor_mul(sq, src[:, ib * 512:(ib + 1) * 512],
                                             src[:, ib * 512:(ib + 1) * 512])
                        nc.tensor.matmul(rn[32 * ib:32 * ib + 2, :],
                                         i2_sb, sq,
                                         start=True, stop=True,
                                         tile_position=(0, 32 * ib))
                    rn_tiles.append(rn)
                # drain rn; q rows scaled by c = 1/(scale_h*log2e)^2 in fp32
                rn_sb = rnp.tile([128, 2, 512], F32, tag="rn")
                nc.vector.tensor_scalar(
                    out=rn_sb[:, 0, :], in0=rn_tiles[0],
                    scalar1=cq_sb[:, p:p + 1], scalar2=None,
                    op0=mybir.AluOpType.mult)
                nc.vector.tensor_copy(rn_sb[:, 1, :], rn_tiles[1])
                # rsqrt on GPSIMD: rr = rn ** -0.5   (fp16 out)
                rr = rrp.tile([128, 2, 512], F16, tag="rr")
                nc.gpsimd.tensor_tensor(
                    out=rr.rearrange("p a b -> p (a b)"),
                    in0=rn_sb.rearrange("p a b -> p (a b)"),
                    in1=nh32, op=POW)

                # bounce rows to DRAM, broadcast to rq_bc / rk_bc
                rbs = []
                for bank in range(2):
                    r_dr = dram.tile([4, 2, 512], F16, tag=f"rd{bank}")
                    for ib in range(4):
                        nc.sync.dma_start(out=r_dr[ib, :, :],
                                          in_=rr[32 * ib:32 * ib + 2, bank, :])
                    r_bc = bcp.tile([128, S], F16, tag=f"rb{bank}")
                    for hh in range(2):
                        col = r_dr[:, hh, :]
                        src = bass.AP(tensor=col.tensor, offset=col.offset,
                                      ap=[[0, 64]] + col.ap)
                        nc.sync.dma_start(
                            out=r_bc[hh * 64:(hh + 1) * 64, :].rearrange(
                                "p (a b) -> p a b", a=4),
                            in_=src)
                    rbs.append(r_bc)

                qs = qsp.tile([128, S], F16, tag="qs")
                nc.vector.tensor_mul(qs, qraw, rbs[0])
                ks = ksp.tile([128, S], F16, tag="ks")
                nc.vector.tensor_mul(ks, kraw, rbs[1])

                return qs, ks, vT

            def build_va(vT):
                # va[p, h, c, 0:64] = v rows for head h, j-chunk c; col 64 = 1
                # One full [128,128] PE transpose per j-chunk gives [j, dh] for
                # both heads at once (head h = cols h*64:(h+1)*64).
                va = vap.tile([128, 2, NJC, 80], F16, tag="va")
                nc.vector.memset(va[:, :, :, 64:65], 1.0)
                for c in range(NJC):
                    tr_ps = work.tile([128, 128], F16, tag="work", name="vtr")
                    nc.tensor.matmul(tr_ps, vT[:, c * 128:(c + 1) * 128],
                                     id_sb, start=True, stop=True,
                                     is_transpose=True)
                    for h in range(2):
                        nc.vector.tensor_copy(va[:, h, c, 0:64],
                                              tr_ps[:, h * 64:(h + 1) * 64])
                return va

            def attention(p, qs, ks, va, scores, epool, stp, cpool, hooks=(),
                          post_t=None):
                ctxTn = ctp.tile([128, 16, 128], F16, tag="ctxTn")
                for ic in range(NIC):
                    for hic, fn in hooks:
                        if ic == hic:
                            fn()
                    i0 = ic * 512
                    # 8 ctxT accums [128, 65] packed 7+1 into 2 banks
                    ctx_ps = cpool.tile([128, 1024], F32, tag="ctx")

                    def pv(jc, e_sb, first, last):
                        for k in range(4):
                            for h in range(2):
                                g = k * 2 + h
                                # start only on the first group per PSUM bank:
                                # start=True zeroing is bank-granular
                                off = ACC_OFF[g]
                                nc.tensor.matmul(
                                    ctx_ps[:, off:off + 65],
                                    e_sb[:, h, k * 128:(k + 1) * 128],
                                    va[:, h, jc, 0:65],
                                    start=(first and g in (0, 7)),
                                    stop=last,
                                    skip_group_check=True)

                    def scores_mm(jc):
                        s_ps = scores.tile([128, 2, 512], F32, tag="s")
                        nc.tensor.matmul(s_ps[:, 0, :],
                                         ks[0:64, jc * 128:(jc + 1) * 128],
                                         qs[0:64, i0:i0 + 512],
                                         start=True, stop=True, tile_position=(0, 0))
                        nc.tensor.matmul(s_ps[:, 1, :],
                                         ks[64:128, jc * 128:(jc + 1) * 128],
                                         qs[64:128, i0:i0 + 512],
                                         start=True, stop=True, tile_position=(64, 0))
                        return s_ps

                    # Pool-share chunks: scores emitted first (their exp runs
                    # on GPSIMD off DVE-staged copies), PV emitted last so the
                    # in-order PE queue never head-blocks on the slow path.
                    pool_es = []
                    for jc in POOL_JCS:
                        s_ps = scores_mm(jc)
                        st = stp.tile([128, 2, 512], F16, tag="st")
                        nc.vector.tensor_copy(
                            st.rearrange("p a b -> p (a b)"),
                            s_ps.rearrange("p a b -> p (a b)"))
                        e_sb = epool.tile([128, 2, 512], F16, tag="e")
                        nc.gpsimd.tensor_tensor(
                            out=e_sb.rearrange("p a b -> p (a b)"),
                            in0=two16,
                            in1=st.rearrange("p a b -> p (a b)"),
                            op=POW)
                        pool_es.append((jc, e_sb))
                    act_jcs = [jc for jc in range(NJC) if jc not in POOL_JCS]
                    for n, jc in enumerate(act_jcs):
                        s_ps = scores_mm(jc)
                        e_sb = epool.tile([128, 2, 512], F16, tag="e")
                        nc.scalar.activation(
                            e_sb.rearrange("p a b -> p (a b)"),
                            s_ps.rearrange("p a b -> p (a b)"),
                            EXP, scale=LN2)
                        pv(jc, e_sb, first=(n == 0), last=False)
                    for n, (jc, e_sb) in enumerate(pool_es):
                        pv(jc, e_sb, first=False, last=(n == len(pool_es) - 1))
                    # normalize: rden = 1/denom-col, drain accums to ctxTn
                    rden = rdp.tile([128, 8], F32, tag="rden")
                    dsrc = bass.AP(tensor=ctx_ps.tensor, offset=ctx_ps.offset + 64,
                                   ap=[ctx_ps.ap[0], [65, 7]])
                    nc.vector.reciprocal(rden[:, 0:7], dsrc)
                    nc.vector.reciprocal(rden[:, 7:8], ctx_ps[:, 576:577])
                    for k in range(4):
                        for h in range(2):
                            off = ACC_OFF[k * 2 + h]
                            nc.vector.tensor_scalar(
                                out=ctxTn[:, ic * 4 + k, h * 64:(h + 1) * 64],
                                in0=ctx_ps[:, off:off + 64],
                                scalar1=rden[:, k * 2 + h:k * 2 + h + 1],
                                scalar2=None,
                                op0=mybir.AluOpType.mult)
                # PE transpose back to [dh, i]
                ctxn = cnp.tile([128, S], F16, tag="ctxn")
                for t in range(16):
                    tr_ps = work.tile([128, 128], F16, tag="work", name="tr")
                    nc.tensor.matmul(tr_ps, ctxTn[:, t, :], id_sb,
                                     start=True, stop=True, is_transpose=True)
                    nc.vector.tensor_copy(ctxn[:, t * 128:(t + 1) * 128], tr_ps)
                    if post_t is not None:
                        post_t(t, ctxn)
                return ctxn

            with tc.tile_pool(name="scores", bufs=2, space="PSUM") as scores, \
                 tc.tile_pool(name="epool", bufs=8) as epool, \
                 tc.tile_pool(name="stp", bufs=2) as stp, \
                 tc.tile_pool(name="cpool", bufs=1, space="PSUM") as cpool:
                pending = {}
                qs0, ks0, vT0 = qkv_and_norms(0)
                pending[0] = (qs0, ks0, build_va(vT0))

                def make_hooks(pn):
                    part = {}

                    def h1():
                        part["qkv"] = qkv_and_norms(pn)

                    def h2():
                        qs_, ks_, vT_ = part["qkv"]
                        pending[pn] = (qs_, ks_, build_va(vT_))
                    return ((1, h1), (3, h2))

                with tc.tile_pool(name="osb", bufs=3) as osb:
                    def outproj_st(st, ctxn2):
                        o_sb = osb.tile([128, D], F16, tag="osb")
                        pairs = [ctxns[0], ctxns[1], ctxn2]
                        for nn in range(2):
                            o_ps = work.tile([128, 512], F32, tag="work",
                                             name="o_ps")
                            for p in range(NPAIR):
                                nc.tensor.matmul(
                                    o_ps[:, 0:384],
                                    pairs[p][:, st * 128:(st + 1) * 128],
                                    wot_sb[:, p, nn * 384:(nn + 1) * 384],
                                    start=(p == 0), stop=(p == NPAIR - 1))
                            nc.vector.tensor_copy(
                                o_sb[:, nn * 384:(nn + 1) * 384], o_ps[:, 0:384])
                        nc.sync.dma_start(out=o[st * 128:(st + 1) * 128, :],
                                          in_=o_sb)

                    for p in range(NPAIR):
                        qs, ks, va = pending.pop(p)
                        hooks = make_hooks(p + 1) if p + 1 < NPAIR else ()
                        post = outproj_st if p == NPAIR - 1 else None
                        ctxns.append(attention(p, qs, ks, va, scores, epool,
                                               stp, cpool, hooks=hooks,
                                               post_t=post))

    nc.compile()
    return nc


def _prep_core_inputs(inputs, b, hg):
    f16 = np.float16
    hs = inputs["hidden_states"]
    rows = slice(hg * 384, (hg + 1) * 384)
    scale6 = np.exp(np.minimum(
        inputs["logit_scale"].reshape(H)[hg * HPC:(hg + 1) * HPC],
        MAX_LOG_SCALE)).astype(np.float64)

    def b3(bias):
        return np.ascontiguousarray(bias[rows].reshape(3, 128).T).astype(np.float32)

    # cq: per-partition scale c = 1/(scale_h*log2e)^2 applied to rn_q
    cq = np.ones((128, 3), np.float32)
    for p in range(3):
        for ib in range(4):
            for hh in range(2):
                cq[32 * ib + hh, p] = 1.0 / (scale6[p * 2 + hh] * LOG2E) ** 2
    i2 = np.zeros((128, 2), f16)
    i2[0:64, 0] = 1.0
    i2[64:128, 1] = 1.0
    return {
        "hst": np.ascontiguousarray(hs[b].T).astype(f16),
        "wqt": np.ascontiguousarray(inputs["Wq"][rows].T).astype(f16),
        "wkt": np.ascontiguousarray(inputs["Wk"][rows].T).astype(f16),
        "wvt": np.ascontiguousarray(inputs["Wv"][rows].T).astype(f16),
        "wot": np.ascontiguousarray(inputs["Wo"][:, rows].T).astype(f16),
        "bq3": b3(inputs["bq"]),
        "bk3": b3(inputs["bk"]),
        "bv3": b3(inputs["bv"]),
        "cq3": cq,
        "i2d": i2,
        "idn": np.eye(128, dtype=f16),
    }


def kernel(**inputs):
    from concourse.bass_utils import run_bass_kernel_spmd
    inputs = {k: np.asarray(v) for k, v in inputs.items()}
    if "nc" not in _NC_CACHE:
        _NC_CACHE["nc"] = build_nc()
    nc = _NC_CACHE["nc"]
    in_maps = [_prep_core_inputs(inputs, c // 2, c % 2) for c in range(8)]
    res = run_bass_kernel_spmd(nc, in_maps, core_ids=list(range(8)))
    out = np.empty((B, S, D), np.float32)
    bo = inputs["bo"].astype(np.float32)
    for b in range(B):
        out[b] = (res.results[2 * b]["o"].astype(np.float32)
                  + res.results[2 * b + 1]["o"].astype(np.float32) + bo)
    return out
